# revision 1
# baseline (speedup 1.0000x reference)
"""Multi-head attention (B=2, S=2048, D=1024, H=16) on 8 Trainium2 cores.

Sharding: head-parallel. Core c owns heads {2c, 2c+1} (a contiguous
128-wide slice of the projection space). Each core reads the full
(transposed, bf16) activations, computes its heads' Q/K/V projections,
full S x S attention, and its partial contribution to the output
projection (row-parallel Wo). Host sums the 8 fp32 partials.

Device-side layout notes:
  - Scores are computed transposed (scoresT[k, q]) so the softmax
    contraction (over k) lands on the PSUM partition axis, where the
    tensor engine can both re-sum it (Z) and contract it with V.
  - K=64 score matmuls are row-tiled two-heads-at-a-time; M=64 attn@V
    matmuls are col-tiled two-heads-at-a-time: full 128x128 PE use.
  - The key-padding mask is a per-partition (per-key) bias of -30000
    applied inside the exp activation (out = exp(in*scale + bias)), so
    masked keys' attention weights are exactly 0 at zero extra cost.
    No max-subtraction pass is needed: scores are ~N(0,1) after the
    1/sqrt(DH) scale, so exp never overflows fp32.
  - The Z (softmax denominator) matmul uses a 64-wide all-ones lhsT,
    which lands Z in PSUM already replicated across each head's 64
    partitions; a single reciprocal + tensor_mul then normalizes both
    heads' O^T in one shot, entirely on the q-partition axis.
"""

import math

import ml_dtypes
import numpy as np

B, S, D, H = 2, 2048, 1024, 16
DH = D // H            # 64
NCORES = 8
MH = 2 * DH            # 128: per-core slice of the head dim (2 heads)
BS = B * S             # 4096
PK = S // 128          # 16 key chunks per batch
PD = D // 128          # 8 contraction chunks for the projections
QT = 512               # q-tile width
NQT = S // QT          # 4 q tiles per batch
SCALE = 1.0 / math.sqrt(DH)
BF16 = ml_dtypes.bfloat16

_NC_CACHE = {}


def _build_nc():
    """Build the (core-independent) Bass program once."""
    if "nc" in _NC_CACHE:
        return _NC_CACHE["nc"]

    from contextlib import ExitStack

    import concourse.bacc as bacc
    import concourse.mybir as mybir
    import concourse.tile as tile

    f32 = mybir.dt.float32
    bf16 = mybir.dt.bfloat16
    Exp = mybir.ActivationFunctionType.Exp

    nc = bacc.Bacc("TRN2", target_bir_lowering=False, debug=False)

    xqT = nc.dram_tensor("xqT", [D, BS], bf16, kind="ExternalInput").ap()
    xkT = nc.dram_tensor("xkT", [D, BS], bf16, kind="ExternalInput").ap()
    xvT = nc.dram_tensor("xvT", [D, BS], bf16, kind="ExternalInput").ap()
    wq = nc.dram_tensor("wq", [128, PD, MH], bf16, kind="ExternalInput").ap()
    wk = nc.dram_tensor("wk", [128, PD, MH], bf16, kind="ExternalInput").ap()
    wv = nc.dram_tensor("wv", [128, PD, MH], bf16, kind="ExternalInput").ap()
    wo = nc.dram_tensor("wo", [128, D], bf16, kind="ExternalInput").ap()
    mb = nc.dram_tensor("mb", [128, B, PK], f32, kind="ExternalInput").ap()
    ident = nc.dram_tensor("ident", [128, 128], bf16, kind="ExternalInput").ap()
    out = nc.dram_tensor("out", [BS, D], f32, kind="ExternalOutput").ap()

    with tile.TileContext(nc) as tc, ExitStack() as ctx:
        wpool = ctx.enter_context(tc.tile_pool(name="wpool", bufs=1))
        apool = ctx.enter_context(tc.tile_pool(name="apool", bufs=1))

        wq_sb = wpool.tile([128, PD, MH], bf16)
        wk_sb = wpool.tile([128, PD, MH], bf16)
        wv_sb = wpool.tile([128, PD, MH], bf16)
        wo_sb = wpool.tile([128, D], bf16)
        mb_sb = wpool.tile([128, B, PK], f32)
        ident_sb = wpool.tile([128, 128], bf16)
        ones_sb = wpool.tile([128, DH], bf16)
        nc.vector.memset(ones_sb, 1.0)
        nc.sync.dma_start(wq_sb, wq)
        nc.sync.dma_start(wk_sb, wk)
        nc.sync.dma_start(wv_sb, wv)
        nc.sync.dma_start(wo_sb, wo)
        nc.sync.dma_start(mb_sb, mb)
        nc.sync.dma_start(ident_sb, ident)

        # Tiny warm-up ops: let DVE/ACT observe the mask DMA early
        # (fewer sync waits on the hot-path instructions) and pull the
        # ~2.7us exp table load off the critical path.
        scratch = wpool.tile([1, 2], f32)
        nc.vector.tensor_copy(scratch, mb_sb[0:1, 0, 0:2])
        scratch2 = wpool.tile([1, 2], f32)
        nc.scalar.activation(scratch2, mb_sb[0:1, 0, 0:2], Exp)

        # Persistent per-core activations:
        #   qT_sb/kT_sb/vT_sb: [128 = 2 heads x 64 head-dims, BS] transposed
        #   v_sb: [128 key positions per chunk, b, chunk, 128 head-dims]
        qT_sb = apool.tile([128, BS], bf16)
        kT_sb = apool.tile([128, BS], bf16)
        vT_sb = apool.tile([128, BS], bf16)
        v_sb = apool.tile([128, B, PK, MH], bf16)

        # ---- Per-batch pipeline: project batch b (2-bank rotating PSUM
        # accumulator over resident per-batch chunk tiles), then run its
        # attention -- so batch 1's projection DMA and matmuls overlap
        # batch 0's ScalarE-bound attention phase.
        xhp = ctx.enter_context(tc.tile_pool(name="xhp", bufs=2))
        with (
            tc.tile_pool(name="atp", bufs=24) as atp,
            tc.tile_pool(name="rp", bufs=3) as rp,
            tc.tile_pool(name="op", bufs=1) as op,
            tc.tile_pool(name="outp", bufs=8) as outp,
            tc.tile_pool(name="psp", bufs=2, space="PSUM") as psp,
            tc.tile_pool(name="pss", bufs=2, space="PSUM") as pss,
            tc.tile_pool(name="psz", bufs=1, space="PSUM") as psz,
            tc.tile_pool(name="pso", bufs=1, space="PSUM") as pso,
        ):
            for b in range(B):
                for xT, w_sb, dst in (
                    (xqT, wq_sb, qT_sb), (xkT, wk_sb, kT_sb), (xvT, wv_sb, vT_sb)
                ):
                    xh = xhp.tile([128, PD, S], bf16, tag="xh")
                    # Two half-width loads per chunk row, first halves first:
                    # the first output slice (and batch 0's attention start)
                    # gates on ~2 MB instead of the full 4 MB half-batch.
                    HW_ = S // 2
                    for half in range(2):
                        for kc in range(PD):
                            cs = b * S + half * HW_
                            nc.sync.dma_start(
                                xh[:, kc, half * HW_:(half + 1) * HW_],
                                xT[kc * 128:(kc + 1) * 128, cs:cs + HW_],
                            )
                    for sti in range(S // QT):
                        pq = psp.tile([128, QT], f32, tag="pq")
                        for kc in range(PD):
                            nc.tensor.matmul(
                                pq,
                                lhsT=w_sb[:, kc, :],
                                rhs=xh[:, kc, sti * QT:(sti + 1) * QT],
                                start=(kc == 0),
                                stop=(kc == PD - 1),
                            )
                        ds = b * S + sti * QT
                        nc.vector.tensor_copy(dst[:, ds:ds + QT], pq)
                # V^T -> V for this batch: DMA-xbar transposes (no PSUM)
                for kci in range(PK):
                    ks = b * S + kci * 128
                    nc.sync.dma_start(
                        v_sb[:, b, kci, :], vT_sb[:, ks:ks + 128], transpose=True
                    )
                otn = op.tile([128, S], bf16, tag="otn")  # normalized O^T (2 heads)
                for qt in range(NQT):
                    qs = b * S + qt * QT
                    zp = psz.tile([128, QT], f32, tag="z")
                    ot = pso.tile([128, QT], f32, tag="ot")
                    for kc in range(PK):
                        ks = b * S + kc * 128
                        sc = pss.tile([128, 2, QT], f32, tag="sc")  # 2 banks
                        # scoresT[k, q] for both heads, row-tiled (K=64 each)
                        nc.tensor.matmul(
                            sc[:, 0, :],
                            lhsT=kT_sb[0:DH, ks:ks + 128],
                            rhs=qT_sb[0:DH, qs:qs + QT],
                        )
                        nc.tensor.matmul(
                            sc[:, 1, :],
                            lhsT=kT_sb[DH:128, ks:ks + 128],
                            rhs=qT_sb[DH:128, qs:qs + QT],
                        )
                        # exp over both heads in one pass (scale = 1/sqrt(DH));
                        # the attn chunk tile dies after its 4 matmuls below.
                        attn = atp.tile([128, 2, QT], bf16, tag="attn")
                        nc.scalar.activation(attn, sc, Exp, scale=SCALE,
                             bias=mb_sb[:, b, kc:kc + 1])
                        # Z (softmax denominator, mask-weighted), replicated
                        # across each head's 64 partitions by the 64-wide lhsT
                        nc.tensor.matmul(
                            zp[0:DH, :], lhsT=ones_sb,
                            rhs=attn[:, 0, :],
                            start=(kc == 0), stop=(kc == PK - 1),
                            skip_group_check=True,
                        )
                        nc.tensor.matmul(
                            zp[DH:128, :], lhsT=ones_sb,
                            rhs=attn[:, 1, :],
                            start=(kc == 0), stop=(kc == PK - 1),
                            skip_group_check=True,
                        )
                        # O^T[d, q] += V[k, d]^T attn[k, q], col-tiled per head
                        nc.tensor.matmul(
                            ot[0:DH, :], lhsT=v_sb[:, b, kc, 0:DH],
                            rhs=attn[:, 0, :],
                            start=(kc == 0), stop=(kc == PK - 1),
                            skip_group_check=True,
                        )
                        nc.tensor.matmul(
                            ot[DH:128, :], lhsT=v_sb[:, b, kc, DH:128],
                            rhs=attn[:, 1, :],
                            start=(kc == 0), stop=(kc == PK - 1),
                            skip_group_check=True,
                        )
                    r = rp.tile([128, QT], f32, tag="r")
                    nc.vector.reciprocal_approx_fast(r, zp)
                    nc.vector.tensor_mul(otn[:, qt * QT:(qt + 1) * QT], ot, r)
                    # Row-parallel output projection for this q-tile's rows:
                    # partial = otn.T @ Wo[mine, :] (K=128, both heads fused)
                    for st in range(qt * 4, qt * 4 + 4):
                        rs = b * S + st * 128
                        wp = pss.tile([128, 2, QT], f32, tag="sc")
                        for nt in range(2):
                            nc.tensor.matmul(
                                wp[:, nt, :],
                                lhsT=otn[:, st * 128:(st + 1) * 128],
                                rhs=wo_sb[:, nt * QT:(nt + 1) * QT],
                            )
                        ws = outp.tile([128, D], f32, tag="ws")
                        nc.vector.tensor_copy(ws, wp)
                        nc.sync.dma_start(out[rs:rs + 128, :], ws)

    nc.compile()
    _NC_CACHE["nc"] = nc
    return nc


def _prep_inputs(queries, keys, values, masks, Wq, Wk, Wv, Wo):
    """Host-side sharding/layout prep. Returns per-core input maps."""
    def t_bf16(x):  # [B, S, D] -> [D, B*S] bf16, contiguous
        return np.ascontiguousarray(
            np.asarray(x, dtype=np.float32).reshape(BS, D).astype(BF16).T
        )

    xqT, xkT, xvT = t_bf16(queries), t_bf16(keys), t_bf16(values)

    m01 = (np.asarray(masks) != 0).astype(np.float32)          # [B, S]
    mb = np.ascontiguousarray(
        np.where(m01.reshape(B, PK, 128) != 0, 0.0, -30000.0)
        .transpose(2, 0, 1).astype(np.float32)
    )

    def w_prep(W, c):  # [D, D] -> [128, PD, MH] bf16 slice for core c
        Wc = np.asarray(W, dtype=np.float32)[:, c * MH:(c + 1) * MH]
        return np.ascontiguousarray(
            Wc.astype(BF16).reshape(PD, 128, MH).transpose(1, 0, 2)
        )

    Wo_f = np.asarray(Wo, dtype=np.float32)
    ident = np.eye(128, dtype=BF16)
    in_maps = []
    for c in range(NCORES):
        in_maps.append({
            "xqT": xqT, "xkT": xkT, "xvT": xvT,
            "wq": w_prep(Wq, c), "wk": w_prep(Wk, c), "wv": w_prep(Wv, c),
            "wo": np.ascontiguousarray(
                Wo_f[c * MH:(c + 1) * MH, :].astype(BF16)
            ),
            "mb": mb, "ident": ident,
        })
    return in_maps


def run(inputs, trace=False, trace_cores=None):
    """Run on 8 NeuronCores; returns (output [B,S,D] f32, BassKernelResults)."""
    from concourse.bass_utils import run_bass_kernel_spmd

    nc = _build_nc()
    in_maps = _prep_inputs(**inputs)
    res = run_bass_kernel_spmd(
        nc, in_maps, core_ids=list(range(NCORES)),
        trace=trace, trace_cores=trace_cores,
    )
    acc = res.results[0]["out"].astype(np.float32, copy=True)
    for r in res.results[1:]:
        acc += r["out"]
    return acc.reshape(B, S, D), res


def kernel(**inputs) -> np.ndarray:
    out, _ = run(inputs)
    return out



# revision 16
# speedup vs baseline: 1.5616x; 1.5616x over previous
"""Multi-head attention (B=2, S=2048, D=1024, H=16) on 8 Trainium2 cores.

Sharding: batch x head-quad. Core c owns batch c//4 and heads
{4g..4g+3} where g = c%4 (a contiguous 256-wide slice of the
projection space). Each core reads its batch's full queries plus the
mask-COMPACTED keys/values (masked-out key positions are dropped on
the host -- exact math, and it halves the attention work and the
key/value traffic), computes its 4 heads' Q/K/V projections, the
S x S_valid attention, and its partial contribution to the output
projection (row-parallel Wo). The host sums the 4 bf16 partials per
batch.

Device-side layout notes:
  - Scores are computed transposed (scoresT[k, q]) so the softmax
    contraction (over k) lands on the PSUM partition axis.
  - The softmax denominator Z is folded into the attn@V matmul: the
    transposed-V tile carries a column of ones adjacent to each head's
    64 head-dims (layout [128, NKV, 132] with ones at cols 1 and 130,
    transposed V pair block at cols 2:130), so each head's attn@V
    lhsT is a contiguous 65-wide slice and Z lands in one PSUM
    partition of the same accumulator -- no separate Z matmuls.
  - 1/Z is then broadcast from that single partition to the head's 64
    partitions with a GPSIMD partition_broadcast (the engine is
    otherwise idle) and applied with one DVE multiply.
  - Key-padding within the last compacted chunk is a per-partition
    bias of -30000 inside the exp activation, so padded lanes produce
    exactly 0. No max-subtraction pass: scores are ~N(0,1) after the
    1/sqrt(DH) scale.
"""

import math

import ml_dtypes
import numpy as np

B, S, D, H = 2, 2048, 1024, 16
DH = D // H            # 64
NCORES = 8
NG = 4                 # head-groups (cores per batch)
NH = H // NG           # 4 heads per core
MH = NH * DH           # 256: per-core slice of the projection space
PD = D // 128          # 8 contraction chunks for the projections
QT = 512               # q-tile width
NQT = S // QT          # 4 q tiles
SCALE = 1.0 / math.sqrt(DH)
BF16 = ml_dtypes.bfloat16

_NC_CACHE = {}


def _build_nc(nkv, dbg=False):
    """Build the (core-independent) Bass program for nkv key chunks."""
    if (nkv, dbg) in _NC_CACHE:
        return _NC_CACHE[(nkv, dbg)]

    from contextlib import ExitStack

    import concourse.bacc as bacc
    import concourse.mybir as mybir
    import concourse.tile as tile

    f32 = mybir.dt.float32
    bf16 = mybir.dt.bfloat16
    Exp = mybir.ActivationFunctionType.Exp

    nkvc = nkv * 128

    nc = bacc.Bacc("TRN2", target_bir_lowering=False, debug=False)

    xq = nc.dram_tensor("xq", [D, S], bf16, kind="ExternalInput").ap()
    xk = nc.dram_tensor("xk", [D, nkvc], bf16, kind="ExternalInput").ap()
    xv = nc.dram_tensor("xv", [D, nkvc], bf16, kind="ExternalInput").ap()
    wq = nc.dram_tensor("wq", [128, PD, 2, 128], bf16, kind="ExternalInput").ap()
    wk = nc.dram_tensor("wk", [128, PD, 2, 128], bf16, kind="ExternalInput").ap()
    wv = nc.dram_tensor("wv", [128, PD, 2, 128], bf16, kind="ExternalInput").ap()
    wo = nc.dram_tensor("wo", [128, 2, D], bf16, kind="ExternalInput").ap()
    mb = nc.dram_tensor("mb", [128, nkv], f32, kind="ExternalInput").ap()
    out = nc.dram_tensor("out", [S, D], bf16, kind="ExternalOutput").ap()
    if dbg:
        qdump = nc.dram_tensor("qdump", [128, 2, S], bf16,
                               kind="ExternalOutput").ap()
        kdump = nc.dram_tensor("kdump", [128, 2, nkv * 128], bf16,
                               kind="ExternalOutput").ap()
        vdump = nc.dram_tensor("vdump", [128, nkv, 2, 2, 66], bf16,
                               kind="ExternalOutput").ap()
        odump = nc.dram_tensor("odump", [128, 2, S], bf16,
                               kind="ExternalOutput").ap()
        otdump = nc.dram_tensor("otdump", [128, QT], f32,
                                kind="ExternalOutput").ap()
        atdump = nc.dram_tensor("atdump", [128, 2, QT], bf16,
                                kind="ExternalOutput").ap()
        rbdump = nc.dram_tensor("rbdump", [64, QT], f32,
                                kind="ExternalOutput").ap()

    # col-tile widths for the K/V projections over nkvc columns
    kv_tiles = []
    cs = 0
    while cs < nkvc:
        w = min(QT, nkvc - cs)
        kv_tiles.append((cs, w))
        cs += w

    with tile.TileContext(nc) as tc, ExitStack() as ctx:
        wpool = ctx.enter_context(tc.tile_pool(name="wpool", bufs=1))
        apool = ctx.enter_context(tc.tile_pool(name="apool", bufs=1))

        wq_sb = wpool.tile([128, PD, 2, 128], bf16)
        wk_sb = wpool.tile([128, PD, 2, 128], bf16)
        wv_sb = wpool.tile([128, PD, 2, 128], bf16)
        wo_sb = wpool.tile([128, 2, D], bf16)
        mb_sb = wpool.tile([128, nkv], f32)
        nc.sync.dma_start(wq_sb, wq)
        nc.sync.dma_start(wk_sb, wk)
        nc.sync.dma_start(wv_sb, wv)
        nc.sync.dma_start(wo_sb, wo)
        nc.sync.dma_start(mb_sb, mb)

        # Tiny warm-up ops: let DVE/ACT observe the mask DMA early and
        # pull the ~2.7us exp table load off the critical path.
        scratch = wpool.tile([1, 2], f32)
        nc.vector.tensor_copy(scratch, mb_sb[0:1, 0:2])
        scratch2 = wpool.tile([1, 2], f32)
        nc.scalar.activation(scratch2, mb_sb[0:1, 0:2], Exp)

        # Persistent per-core activations (partition = 2x64 head dims of
        # a head pair; the pair index is the second axis).
        xq_sb = apool.tile([128, PD, S], bf16)
        xk_sb = apool.tile([128, PD, nkvc], bf16)
        xv_sb = apool.tile([128, PD, nkvc], bf16)
        qT_sb = apool.tile([128, 2, S], bf16)
        kT_sb = apool.tile([128, 2, nkvc], bf16)
        vT_sb = apool.tile([128, 2, nkvc], bf16)
        # attn@V lhsT: per (pair, head) a 66-col sub-block [d0..d63, 1,
        # pad], so each head's 65-wide lhsT slice [V | 1] is contiguous
        # and Z lands on (aligned) PSUM partition 64.
        v65_sb = apool.tile([128, nkv, 2, 2, 66], bf16)
        nc.vector.memset(v65_sb, 1.0)
        otn_sb = apool.tile([128, 2, S], bf16)

        with (
            tc.tile_pool(name="atp", bufs=16) as atp,
            tc.tile_pool(name="rp", bufs=4) as rp,
            tc.tile_pool(name="rbp", bufs=4) as rbp,
            tc.tile_pool(name="outp", bufs=8) as outp,
            tc.tile_pool(name="pp4", bufs=4, space="PSUM") as pp4,
            tc.tile_pool(name="pss", bufs=2, space="PSUM") as pss,
        ):
            # Input DMAs: queries first (half-column loads so the first
            # projection gates on ~2 MB), then compacted keys/values.
            HW_ = S // 2
            for half in range(2):
                for kc in range(PD):
                    nc.sync.dma_start(
                        xq_sb[:, kc, half * HW_:(half + 1) * HW_],
                        xq[kc * 128:(kc + 1) * 128,
                           half * HW_:(half + 1) * HW_],
                    )
            for kc in range(PD):
                nc.sync.dma_start(xk_sb[:, kc, :], xk[kc * 128:(kc + 1) * 128, :])
            for kc in range(PD):
                nc.sync.dma_start(xv_sb[:, kc, :], xv[kc * 128:(kc + 1) * 128, :])

            # Projections: dst^T[t*128 + m, col] over 128-col chunks of D
            for x_sb, w_sb, dst, tiles in (
                (xq_sb, wq_sb, qT_sb, [(i * QT, QT) for i in range(NQT)]),
                (xk_sb, wk_sb, kT_sb, kv_tiles),
                (xv_sb, wv_sb, vT_sb, kv_tiles),
            ):
                for t in range(2):
                    for (cs, w) in tiles:
                        pq = pp4.tile([128, QT], f32, tag="pq")
                        for kc in range(PD):
                            nc.tensor.matmul(
                                pq[:, 0:w],
                                lhsT=w_sb[:, kc, t, :],
                                rhs=x_sb[:, kc, cs:cs + w],
                                start=(kc == 0),
                                stop=(kc == PD - 1),
                            )
                        nc.vector.tensor_copy(dst[:, t, cs:cs + w], pq[:, 0:w])

            # V^T -> V65: DMA-xbar transpose of each [128,128] pair
            # block into a contiguous temp (the transpose engine can't
            # write strided or partition-offset blocks), then one DVE
            # copy splits it across the two 66-col head sub-blocks.
            with tc.tile_pool(name="vtp", bufs=4) as vtp:
                for kc in range(nkv):
                    for t in range(2):
                        tmp = vtp.tile([128, 128], bf16, tag="vt")
                        nc.sync.dma_start(
                            tmp, vT_sb[:, t, kc * 128:(kc + 1) * 128],
                            transpose=True,
                        )
                        nc.vector.tensor_copy(v65_sb[:, kc, t, :, 0:64], tmp)

            for qt in range(NQT):
                qs = qt * QT
                for t in range(2):  # head pair
                    ota = pp4.tile([128, QT], f32, tag="pq")
                    otb = pp4.tile([128, QT], f32, tag="pq")
                    for kc in range(nkv):
                        ks = kc * 128
                        sc = pss.tile([128, 2, QT], f32, tag="sc")
                        nc.tensor.matmul(
                            sc[:, 0, :],
                            lhsT=kT_sb[0:DH, t, ks:ks + 128],
                            rhs=qT_sb[0:DH, t, qs:qs + QT],
                        )
                        nc.tensor.matmul(
                            sc[:, 1, :],
                            lhsT=kT_sb[DH:128, t, ks:ks + 128],
                            rhs=qT_sb[DH:128, t, qs:qs + QT],
                        )
                        attn = atp.tile([128, 2, QT], bf16, tag="attn")
                        nc.scalar.activation(attn, sc, Exp, scale=SCALE,
                                             bias=mb_sb[:, kc:kc + 1])
                        if dbg and qt == 0 and t == 0 and kc == 0:
                            nc.sync.dma_start(atdump, attn)
                        # attn@V with Z folded in: head even's lhsT is
                        # [1|V], so ota[0]=Z, ota[1:65]=O^T; head odd's
                        # is [V|1], so otb[0:64]=O^T, otb[64]=Z.
                        nc.tensor.matmul(
                            ota[0:65, :], lhsT=v65_sb[:, kc, t, 0, 0:65],
                            rhs=attn[:, 0, :],
                            start=(kc == 0), stop=(kc == nkv - 1),
                            skip_group_check=True,
                        )
                        nc.tensor.matmul(
                            otb[0:65, :], lhsT=v65_sb[:, kc, t, 1, 0:65],
                            rhs=attn[:, 1, :],
                            start=(kc == 0), stop=(kc == nkv - 1),
                            skip_group_check=True,
                        )
                    if dbg and qt == 0 and t == 0:
                        otsb = rp.tile([128, QT], f32, tag="otd")
                        nc.vector.tensor_copy(otsb, ota)
                        nc.sync.dma_start(otdump, otsb)
                    for hodd, ot in ((0, ota), (1, otb)):
                        # Z row sits at PSUM partition 64; custom-DVE ops
                        # mislower partition-offset reads, so stage it to
                        # partition 0 of SBUF with a plain copy first.
                        zc = rp.tile([1, QT], f32, tag="zc")
                        nc.vector.tensor_copy(zc, ot[64:65, :])
                        r = rp.tile([1, QT], f32, tag="r")
                        nc.vector.reciprocal_approx_fast(r, zc)
                        rb = rbp.tile([64, QT], f32, tag="rb")
                        nc.gpsimd.partition_broadcast(rb, r, channels=64)
                        if dbg and qt == 0 and t == 0 and hodd == 0:
                            nc.sync.dma_start(rbdump, rb)
                        nc.vector.tensor_mul(
                            otn_sb[hodd * 64:hodd * 64 + 64, t, qs:qs + QT],
                            ot[0:64, :], rb)
                # Row-parallel output projection for this q-tile's rows
                for st in range(qt * 4, qt * 4 + 4):
                    rs = st * 128
                    wp = pss.tile([128, 2, QT], f32, tag="sc")
                    for nt in range(2):
                        for t in range(2):
                            nc.tensor.matmul(
                                wp[:, nt, :],
                                lhsT=otn_sb[:, t, rs:rs + 128],
                                rhs=wo_sb[:, t, nt * QT:(nt + 1) * QT],
                                start=(t == 0), stop=(t == 1),
                            )
                    ws = outp.tile([128, D], bf16, tag="ws")
                    nc.vector.tensor_copy(ws, wp)
                    nc.sync.dma_start(out[rs:rs + 128, :], ws)

            if dbg:
                nc.sync.dma_start(qdump, qT_sb)
                nc.sync.dma_start(kdump, kT_sb)
                nc.sync.dma_start(vdump, v65_sb)
                nc.sync.dma_start(odump, otn_sb)

    nc.compile()
    _NC_CACHE[(nkv, dbg)] = nc
    return nc


def _prep_inputs(queries, keys, values, masks, Wq, Wk, Wv, Wo):
    """Host-side sharding/layout prep. Returns (nkv, per-core input maps)."""
    masks = np.asarray(masks)
    sels = [np.nonzero(masks[b] != 0)[0] for b in range(B)]
    ns = [len(s) for s in sels]
    nkv = max(1, (max(ns) + 127) // 128)
    nkvc = nkv * 128

    def t_bf16(x):  # [S, D] f32 -> [D, S] bf16 contiguous
        return np.ascontiguousarray(
            np.asarray(x, dtype=np.float32).astype(BF16).T)

    def t_comp(x, sel):  # gather valid rows, pad to nkvc, transpose
        xc = np.zeros((nkvc, D), dtype=BF16)
        xc[:len(sel)] = np.asarray(x, dtype=np.float32)[sel].astype(BF16)
        return np.ascontiguousarray(xc.T)

    xq_b = [t_bf16(queries[b]) for b in range(B)]
    xk_b = [t_comp(keys[b], sels[b]) for b in range(B)]
    xv_b = [t_comp(values[b], sels[b]) for b in range(B)]

    mb_b = []
    for b in range(B):
        j = np.arange(nkvc).reshape(nkv, 128)
        mb_b.append(np.ascontiguousarray(
            np.where(j < ns[b], 0.0, -30000.0).astype(np.float32).T))

    def w_prep(W, g):  # [D, D] -> [128, PD, 2, 128] bf16 slice for group g
        Wg = np.asarray(W, dtype=np.float32)[:, g * MH:(g + 1) * MH]
        return np.ascontiguousarray(
            Wg.astype(BF16).reshape(PD, 128, 2, 128).transpose(1, 0, 2, 3))

    def wo_prep(W, g):  # [D, D] -> [128, 2, D] bf16 slice for group g
        Wg = np.asarray(W, dtype=np.float32)[g * MH:(g + 1) * MH, :]
        return np.ascontiguousarray(
            Wg.astype(BF16).reshape(2, 128, D).transpose(1, 0, 2))

    wq_g = [w_prep(Wq, g) for g in range(NG)]
    wk_g = [w_prep(Wk, g) for g in range(NG)]
    wv_g = [w_prep(Wv, g) for g in range(NG)]
    wo_g = [wo_prep(Wo, g) for g in range(NG)]

    in_maps = []
    for c in range(NCORES):
        b, g = c // NG, c % NG
        in_maps.append({
            "xq": xq_b[b], "xk": xk_b[b], "xv": xv_b[b],
            "wq": wq_g[g], "wk": wk_g[g], "wv": wv_g[g], "wo": wo_g[g],
            "mb": mb_b[b],
        })
    return nkv, in_maps


def run(inputs, trace=False, trace_cores=None):
    """Run on 8 NeuronCores; returns (output [B,S,D] f32, BassKernelResults)."""
    from concourse.bass_utils import run_bass_kernel_spmd

    nkv, in_maps = _prep_inputs(**inputs)
    nc = _build_nc(nkv)
    res = run_bass_kernel_spmd(
        nc, in_maps, core_ids=list(range(NCORES)),
        trace=trace, trace_cores=trace_cores,
    )
    out = np.empty((B, S, D), dtype=np.float32)
    for b in range(B):
        acc = res.results[b * NG]["out"].astype(np.float32)
        for g in range(1, NG):
            acc += res.results[b * NG + g]["out"].astype(np.float32)
        out[b] = acc
    return out, res


def kernel(**inputs) -> np.ndarray:
    out, _ = run(inputs)
    return out


# revision 46
# speedup vs baseline: 1.9639x; 1.2576x over previous
"""Multi-head attention (B=2, S=2048, D=1024, H=16) on 8 Trainium2 cores.

Sharding: batch x head-quad. Core c owns batch c//4 and heads
{4g..4g+3} where g = c%4 (a contiguous 256-wide slice of the
projection space). Each core reads its batch's full queries plus the
mask-COMPACTED keys/values (masked-out key positions are dropped on
the host -- exact math, and it halves the attention work and the
key/value traffic), computes its 4 heads' Q/K/V projections, the
S x S_valid attention, and its partial contribution to the output
projection (row-parallel Wo). The host sums the 4 bf16 partials per
batch.

Device-side layout notes:
  - Scores are computed transposed (scoresT[k, q]) so the softmax
    contraction (over k) lands on the PSUM partition axis.
  - The softmax denominator Z is folded into the attn@V matmul: the
    transposed-V tile carries a column of ones adjacent to each head's
    64 head-dims (layout [128, NKV, 132] with ones at cols 1 and 130,
    transposed V pair block at cols 2:130), so each head's attn@V
    lhsT is a contiguous 65-wide slice and Z lands in one PSUM
    partition of the same accumulator -- no separate Z matmuls.
  - 1/Z is then broadcast from that single partition to the head's 64
    partitions with a GPSIMD partition_broadcast (the engine is
    otherwise idle) and applied with one DVE multiply.
  - Key-padding within the last compacted chunk is a per-partition
    bias of -30000 inside the exp activation, so padded lanes produce
    exactly 0. No max-subtraction pass: scores are ~N(0,1) after the
    1/sqrt(DH) scale.
"""

import math

import ml_dtypes
import numpy as np

B, S, D, H = 2, 2048, 1024, 16
DH = D // H            # 64
NCORES = 8
NG = 4                 # head-groups (cores per batch)
NH = H // NG           # 4 heads per core
MH = NH * DH           # 256: per-core slice of the projection space
PD = D // 128          # 8 contraction chunks for the projections
QT = 512               # q-tile width
NQT = S // QT          # 4 q tiles
SCALE = 1.0 / math.sqrt(DH)
BF16 = ml_dtypes.bfloat16

_NC_CACHE = {}


def _build_nc(nkv, dbg=False):
    """Build the (core-independent) Bass program for nkv key chunks."""
    if (nkv, dbg) in _NC_CACHE:
        return _NC_CACHE[(nkv, dbg)]

    from contextlib import ExitStack

    import concourse.bacc as bacc
    import concourse.mybir as mybir
    import concourse.tile as tile

    f32 = mybir.dt.float32
    bf16 = mybir.dt.bfloat16
    Exp = mybir.ActivationFunctionType.Exp

    nkvc = nkv * 128

    nc = bacc.Bacc("TRN2", target_bir_lowering=False, debug=False)

    xq = nc.dram_tensor("xq", [128, PD, S], bf16, kind="ExternalInput").ap()
    xk = nc.dram_tensor("xk", [128, PD, nkvc], bf16, kind="ExternalInput").ap()
    xv = nc.dram_tensor("xv", [128, PD, nkvc], bf16, kind="ExternalInput").ap()
    wq = nc.dram_tensor("wq", [128, PD, 2, 128], bf16, kind="ExternalInput").ap()
    wk = nc.dram_tensor("wk", [128, PD, 2, 128], bf16, kind="ExternalInput").ap()
    wv = nc.dram_tensor("wv", [128, PD, 2, 128], bf16, kind="ExternalInput").ap()
    wo = nc.dram_tensor("wo", [128, 2, D], bf16, kind="ExternalInput").ap()
    mb = nc.dram_tensor("mb", [128, nkv], f32, kind="ExternalInput").ap()
    # out[p, st, :] holds output row st*128 + p
    out = nc.dram_tensor("out", [128, NQT * 4, D], bf16,
                         kind="ExternalOutput").ap()
    if dbg:
        qdump = nc.dram_tensor("qdump", [128, 2, S], bf16,
                               kind="ExternalOutput").ap()
        kdump = nc.dram_tensor("kdump", [128, 2, nkv * 128], bf16,
                               kind="ExternalOutput").ap()
        vdump = nc.dram_tensor("vdump", [128, nkv, 2, 2, 66], bf16,
                               kind="ExternalOutput").ap()
        odump = nc.dram_tensor("odump", [128, 2, S], bf16,
                               kind="ExternalOutput").ap()
        otdump = nc.dram_tensor("otdump", [128, QT], f32,
                                kind="ExternalOutput").ap()
        atdump = nc.dram_tensor("atdump", [128, 2, QT], bf16,
                                kind="ExternalOutput").ap()
        rbdump = nc.dram_tensor("rbdump", [64, QT], f32,
                                kind="ExternalOutput").ap()

    # col-tile widths for the K/V projections over nkvc columns
    kv_tiles = []
    cs = 0
    while cs < nkvc:
        w = min(QT, nkvc - cs)
        kv_tiles.append((cs, w))
        cs += w

    with tile.TileContext(nc) as tc, ExitStack() as ctx:
        wpool = ctx.enter_context(tc.tile_pool(name="wpool", bufs=1))
        apool = ctx.enter_context(tc.tile_pool(name="apool", bufs=1))

        wq_sb = wpool.tile([128, PD, 2, 128], bf16)
        wk_sb = wpool.tile([128, PD, 2, 128], bf16)
        wv_sb = wpool.tile([128, PD, 2, 128], bf16)
        wo_sb = wpool.tile([128, 2, D], bf16)
        mb_sb = wpool.tile([128, nkv], f32)
        nc.sync.dma_start(wq_sb, wq)
        nc.sync.dma_start(mb_sb, mb)

        # Tiny warm-up ops: let DVE/ACT observe the mask DMA early and
        # pull the ~2.7us exp table load off the critical path.
        scratch = wpool.tile([1, 2], f32)
        nc.vector.tensor_copy(scratch, mb_sb[0:1, 0:2])
        scratch2 = wpool.tile([1, 2], f32)
        nc.scalar.activation(scratch2, mb_sb[0:1, 0:2], Exp)

        # Persistent per-core activations (partition = 2x64 head dims of
        # a head pair; the pair index is the second axis).
        xq_sb = apool.tile([128, PD, S], bf16)
        xk_sb = apool.tile([128, PD, nkvc], bf16)
        xv_sb = apool.tile([128, PD, nkvc], bf16)
        qT_sb = apool.tile([128, 2, S], bf16)
        kT_sb = apool.tile([128, 2, nkvc], bf16)
        # attn@V lhsT: per (pair, head) a 66-col sub-block [d0..d63, 1,
        # pad], so each head's 65-wide lhsT slice [V | 1] is contiguous
        # and Z lands on (aligned) PSUM partition 64.
        v65_sb = apool.tile([128, nkv, 2, 2, 66], bf16)
        nc.vector.memset(v65_sb, 1.0)
        otn_sb = apool.tile([128, 2, S], bf16)

        with (
            tc.tile_pool(name="atp", bufs=16) as atp,
            tc.tile_pool(name="rp", bufs=4) as rp,
            tc.tile_pool(name="rbp", bufs=4) as rbp,
            tc.tile_pool(name="outp", bufs=2) as outp,
            tc.tile_pool(name="pot", bufs=2, space="PSUM") as pot,
            tc.tile_pool(name="paux", bufs=2, space="PSUM") as paux,
            tc.tile_pool(name="pss", bufs=2, space="PSUM") as pss,
        ):
            # Input DMAs in consumption order, column-blocked so early
            # projection groups unblock as soon as their bytes land. One
            # coalesced DMA per (tensor, col-block): the [128, PD, cols]
            # DRAM layout matches the SBUF staging layout, so a whole
            # block moves in a single (cheap) HWDGE job.
            def dma_blk(x_sb, xd, cs, w):
                nc.sync.dma_start(x_sb[:, :, cs:cs + w], xd[:, :, cs:cs + w])

            dma_blk(xq_sb, xq, 0, QT)
            nc.sync.dma_start(wk_sb, wk)
            nc.sync.dma_start(wv_sb, wv)
            dma_blk(xk_sb, xk, *kv_tiles[0])
            dma_blk(xv_sb, xv, *kv_tiles[0])
            if len(kv_tiles) > 1:
                dma_blk(xk_sb, xk, *kv_tiles[1])
                dma_blk(xv_sb, xv, *kv_tiles[1])
            dma_blk(xq_sb, xq, QT, QT)
            for (cs, w) in kv_tiles[2:]:
                dma_blk(xk_sb, xk, cs, w)
                dma_blk(xv_sb, xv, cs, w)
            dma_blk(xq_sb, xq, 2 * QT, QT)
            dma_blk(xq_sb, xq, 3 * QT, QT)
            nc.sync.dma_start(wo_sb, wo)

            def proj(x_sb, w_sb, dst, t, cs, w):
                pq = paux.tile([128, QT], f32, tag="pq")
                for kc in range(PD):
                    nc.tensor.matmul(
                        pq[:, 0:w],
                        lhsT=w_sb[:, kc, t, :],
                        rhs=x_sb[:, kc, cs:cs + w],
                        start=(kc == 0),
                        stop=(kc == PD - 1),
                    )
                nc.vector.tensor_copy(dst[:, t, cs:cs + w], pq[:, 0:w])

            def projv(kc):
                # V projection for one 128-wide kv chunk, directly in the
                # [kvpos, dims] orientation (lhsT = the xv chunk itself),
                # so no transpose is needed: one DVE copy per pair drops
                # the [128, 128] pair block into the 66-col sub-blocks.
                pq = paux.tile([128, QT], f32, tag="pq")
                ks = kc * 128
                for kcin in range(PD):
                    nc.tensor.matmul(
                        pq[:, 0:MH],
                        lhsT=xv_sb[:, kcin, ks:ks + 128],
                        rhs=wv_sb[:, kcin, :, :],
                        start=(kcin == 0),
                        stop=(kcin == PD - 1),
                    )
                for t in range(2):
                    nc.vector.tensor_copy(
                        v65_sb[:, kc, t, :, 0:64],
                        pq[:, t * 128:(t + 1) * 128])

            ws4_box = [None]

            def emit_outproj_st(st, fine_dma=False):
                # Row-parallel output projection for one 128-row slice;
                # rows collect in a per-q-tile [128, 4, D] staging tile
                # that ships in one coalesced DMA (or per-slice DMAs for
                # the trailing q-tile, to shorten the kernel tail).
                rs = st * 128
                if st % 4 == 0:
                    ws4_box[0] = outp.tile([128, 4, D], bf16, tag="ws",
                                           name="ws4")
                ws4 = ws4_box[0]
                for nt in range(2):
                    wp = paux.tile([128, QT], f32, tag="pq")
                    for tt in range(2):
                        nc.tensor.matmul(
                            wp,
                            lhsT=otn_sb[:, tt, rs:rs + 128],
                            rhs=wo_sb[:, tt, nt * QT:(nt + 1) * QT],
                            start=(tt == 0), stop=(tt == 1),
                        )
                    nc.vector.tensor_copy(
                        ws4[:, st % 4, nt * QT:(nt + 1) * QT], wp)
                if fine_dma:
                    nc.sync.dma_start(out[:, st:st + 1, :],
                                      ws4[:, st % 4:st % 4 + 1, :])
                elif st % 4 == 3:
                    nc.sync.dma_start(out[:, st - 3:st + 1, :], ws4)

            # Pair-phased pipeline: a minimal prologue projects just what
            # the first attention chunks need; every other projection
            # group (pair 0 remainder, V chunks, then all of pair 1) is
            # drip-fed into the attention chunk stream as a PE filler, so
            # the (ACT-bound) exp stream starts as early as the input
            # DMAs allow and the PE stays fed. Pair 1's attention carries
            # the deferred output projections the same way, so the PE
            # never waits on a q-tile's normalize chain.
            proj(xq_sb, wq_sb, qT_sb, 0, 0, QT)
            proj(xk_sb, wk_sb, kT_sb, 0, *kv_tiles[0])
            projv(0)

            def F(f, *a):
                return lambda: f(*a)

            # Filler slots are global chunk indices; a slot's fillers are
            # emitted between that chunk's score and attn@V matmuls, so a
            # filler at slot c runs before av(chunk c) and before
            # sc(chunk c+1). Deadlines (all within pair-0 qt0):
            #   projv(kc): slot kc (just-in-time for its own av)
            #   k-proj col-tile j (cols cs..): slot cs//128 - 1
            #   q sti_i: slot i*nkv - 1 (first score of q-tile i)
            fill_at = {}

            def add_fill(slot, f):
                fill_at.setdefault(slot, []).append(f)

            for kc in range(1, nkv):
                add_fill(kc, F(projv, kc))
            for (cs, w) in kv_tiles[1:]:
                add_fill(cs // 128 - 1,
                         F(proj, xk_sb, wk_sb, kT_sb, 0, cs, w))
            for sti in range(1, NQT):
                add_fill(sti * nkv - 1,
                         F(proj, xq_sb, wq_sb, qT_sb, 0, sti * QT, QT))
            # pair-1 projections: anywhere before chunk 4*nkv (and their
            # own q-tile deadlines, which these early slots satisfy)
            t1_fills = (
                [F(proj, xk_sb, wk_sb, kT_sb, 1, cs, w) for (cs, w) in kv_tiles]
                + [F(proj, xq_sb, wq_sb, qT_sb, 1, sti * QT, QT)
                   for sti in range(NQT)]
            )
            for j, f in enumerate(t1_fills):
                add_fill(nkv + 2 + 2 * j, f)

            nchunk = 0
            # 4 evenly spread chunk indices at which to emit one deferred
            # output-projection slice during pair 1's attention
            op_marks = [1 + (j * (nkv - 2)) // 3 for j in range(4)]
            assert len(set(op_marks)) == 4
            for t in range(2):
                for qt in range(NQT):
                    qs = qt * QT
                    ota = pot.tile([128, QT], f32, tag="ot")
                    otb = pot.tile([128, QT], f32, tag="ot")
                    for kc in range(nkv):
                        ks = kc * 128
                        sc = pss.tile([128, 2, QT], f32, tag="sc")
                        nc.tensor.matmul(
                            sc[:, 0, :],
                            lhsT=kT_sb[0:DH, t, ks:ks + 128],
                            rhs=qT_sb[0:DH, t, qs:qs + QT],
                        )
                        nc.tensor.matmul(
                            sc[:, 1, :],
                            lhsT=kT_sb[DH:128, t, ks:ks + 128],
                            rhs=qT_sb[DH:128, t, qs:qs + QT],
                        )
                        attn = atp.tile([128, 2, QT], bf16, tag="attn")
                        nc.scalar.activation(attn, sc, Exp, scale=SCALE,
                                             bias=mb_sb[:, kc:kc + 1])
                        if dbg and qt == 0 and t == 0 and kc == 0:
                            nc.sync.dma_start(atdump, attn)
                        # PE fillers sit between the score and attn@V
                        # matmuls so they hide inside the exp wait
                        # instead of delaying the next chunk's scores:
                        # pair-1 projection groups during pair 0's
                        # attention, deferred output projections during
                        # pair 1's.
                        if t == 0:
                            for f in fill_at.get(nchunk, ()):
                                f()
                        elif qt >= 1 and kc in op_marks:
                            emit_outproj_st(
                                (qt - 1) * 4 + op_marks.index(kc))
                        nchunk += 1
                        # attn@V with Z folded in via the ones column:
                        # O^T in rows 0:64, Z in row 64.
                        nc.tensor.matmul(
                            ota[0:65, :], lhsT=v65_sb[:, kc, t, 0, 0:65],
                            rhs=attn[:, 0, :],
                            start=(kc == 0), stop=(kc == nkv - 1),
                            skip_group_check=True,
                        )
                        nc.tensor.matmul(
                            otb[0:65, :], lhsT=v65_sb[:, kc, t, 1, 0:65],
                            rhs=attn[:, 1, :],
                            start=(kc == 0), stop=(kc == nkv - 1),
                            skip_group_check=True,
                        )

                    if dbg and qt == 0 and t == 0:
                        otsb = rp.tile([128, QT], f32, tag="otd")
                        nc.vector.tensor_copy(otsb, ota)
                        nc.sync.dma_start(otdump, otsb)
                    for hodd, ot in ((0, ota), (1, otb)):
                        # Z row sits at PSUM partition 64; custom-DVE ops
                        # mislower partition-offset reads, so stage it to
                        # partition 0 of SBUF with a plain copy first.
                        zc = rp.tile([1, QT], f32, tag="zc")
                        nc.vector.tensor_copy(zc, ot[64:65, :])
                        r = rp.tile([1, QT], f32, tag="r")
                        nc.vector.reciprocal_approx_fast(r, zc)
                        rb = rbp.tile([64, QT], f32, tag="rb")
                        nc.gpsimd.partition_broadcast(rb, r, channels=64)
                        if dbg and qt == 0 and t == 0 and hodd == 0:
                            nc.sync.dma_start(rbdump, rb)
                        nc.vector.tensor_mul(
                            otn_sb[hodd * 64:hodd * 64 + 64, t, qs:qs + QT],
                            ot[0:64, :], rb)
                # leftover fillers at end of pair-0 phase (slots past
                # the last pair-0 chunk, e.g. when nkv is small)
                if t == 0 and qt == NQT - 1:
                    for slot in sorted(fill_at):
                        if slot >= nchunk:
                            for f in fill_at[slot]:
                                f()
            for st in range((NQT - 1) * 4, NQT * 4):
                emit_outproj_st(st, fine_dma=True)

            if dbg:
                nc.sync.dma_start(qdump, qT_sb)
                nc.sync.dma_start(kdump, kT_sb)
                nc.sync.dma_start(vdump, v65_sb)
                nc.sync.dma_start(odump, otn_sb)

    nc.compile()
    _NC_CACHE[(nkv, dbg)] = nc
    return nc


def _prep_inputs(queries, keys, values, masks, Wq, Wk, Wv, Wo):
    """Host-side sharding/layout prep. Returns (nkv, per-core input maps)."""
    masks = np.asarray(masks)
    sels = [np.nonzero(masks[b] != 0)[0] for b in range(B)]
    ns = [len(s) for s in sels]
    nkv = max(1, (max(ns) + 127) // 128)
    nkvc = nkv * 128

    def t_bf16(x):  # [S, D] f32 -> [128, PD, S] bf16 contiguous
        xt = np.asarray(x, dtype=np.float32).astype(BF16).T  # [D, S]
        return np.ascontiguousarray(
            xt.reshape(PD, 128, -1).transpose(1, 0, 2))

    def t_comp(x, sel):  # gather valid rows, pad to nkvc, relayout
        xc = np.zeros((nkvc, D), dtype=BF16)
        xc[:len(sel)] = np.asarray(x, dtype=np.float32)[sel].astype(BF16)
        return np.ascontiguousarray(
            xc.T.reshape(PD, 128, nkvc).transpose(1, 0, 2))

    xq_b = [t_bf16(queries[b]) for b in range(B)]
    xk_b = [t_comp(keys[b], sels[b]) for b in range(B)]
    xv_b = [t_comp(values[b], sels[b]) for b in range(B)]

    mb_b = []
    for b in range(B):
        j = np.arange(nkvc).reshape(nkv, 128)
        mb_b.append(np.ascontiguousarray(
            np.where(j < ns[b], 0.0, -30000.0).astype(np.float32).T))

    def w_prep(W, g):  # [D, D] -> [128, PD, 2, 128] bf16 slice for group g
        Wg = np.asarray(W, dtype=np.float32)[:, g * MH:(g + 1) * MH]
        return np.ascontiguousarray(
            Wg.astype(BF16).reshape(PD, 128, 2, 128).transpose(1, 0, 2, 3))

    def wo_prep(W, g):  # [D, D] -> [128, 2, D] bf16 slice for group g
        Wg = np.asarray(W, dtype=np.float32)[g * MH:(g + 1) * MH, :]
        return np.ascontiguousarray(
            Wg.astype(BF16).reshape(2, 128, D).transpose(1, 0, 2))

    wq_g = [w_prep(Wq, g) for g in range(NG)]
    wk_g = [w_prep(Wk, g) for g in range(NG)]
    wv_g = [w_prep(Wv, g) for g in range(NG)]
    wo_g = [wo_prep(Wo, g) for g in range(NG)]

    in_maps = []
    for c in range(NCORES):
        b, g = c // NG, c % NG
        in_maps.append({
            "xq": xq_b[b], "xk": xk_b[b], "xv": xv_b[b],
            "wq": wq_g[g], "wk": wk_g[g], "wv": wv_g[g], "wo": wo_g[g],
            "mb": mb_b[b],
        })
    return nkv, in_maps


def run(inputs, trace=False, trace_cores=None):
    """Run on 8 NeuronCores; returns (output [B,S,D] f32, BassKernelResults)."""
    from concourse.bass_utils import run_bass_kernel_spmd

    nkv, in_maps = _prep_inputs(**inputs)
    nc = _build_nc(nkv)
    res = run_bass_kernel_spmd(
        nc, in_maps, core_ids=list(range(NCORES)),
        trace=trace, trace_cores=trace_cores,
    )
    out = np.empty((B, S, D), dtype=np.float32)
    for b in range(B):
        acc = res.results[b * NG]["out"].astype(np.float32)
        for g in range(1, NG):
            acc += res.results[b * NG + g]["out"].astype(np.float32)
        # [128, 16, D], row st*128+p at [p, st] -> [S, D]
        out[b] = acc.transpose(1, 0, 2).reshape(S, D)
    return out, res


def kernel(**inputs) -> np.ndarray:
    out, _ = run(inputs)
    return out


# revision 53
# speedup vs baseline: 1.9788x; 1.0076x over previous
"""Multi-head attention (B=2, S=2048, D=1024, H=16) on 8 Trainium2 cores.

Sharding: batch x head-quad. Core c owns batch c//4 and heads
{4g..4g+3} where g = c%4 (a contiguous 256-wide slice of the
projection space). Each core reads its batch's full queries plus the
mask-COMPACTED keys/values (masked-out key positions are dropped on
the host -- exact math, and it halves the attention work and the
key/value traffic), computes its 4 heads' Q/K/V projections, the
S x S_valid attention, and its partial contribution to the output
projection (row-parallel Wo). The host sums the 4 bf16 partials per
batch.

Device-side layout notes:
  - Scores are computed transposed (scoresT[k, q]) so the softmax
    contraction (over k) lands on the PSUM partition axis. The two
    K=64 score matmuls of a head pair sit on SBUF partitions 0:64 and
    64:128, so bass's inferred tile_position row-tiles them into
    concurrent halves of the PE array on hardware.
  - V is projected directly in [kvpos, dims] orientation (lhsT = the
    xv chunk itself), so no transposes are needed anywhere.
  - The softmax denominator Z is folded into the attn@V matmul: each
    head's lhsT is a contiguous 65-wide [V | 1] slice (66-col
    sub-blocks of the v65 tile), so Z lands on (aligned) PSUM
    partition 64 of the same accumulator -- no separate Z matmuls.
    1/Z is broadcast to the head's 64 partitions with a GPSIMD
    partition_broadcast (the engine is otherwise idle) and applied
    with one DVE multiply, after a DVE evacuation of the accumulator
    to SBUF that frees the PSUM bank for the next q-tile.
  - The whole program is software-pipelined around the ACT-bound exp
    stream: a minimal prologue, then every projection group and the
    deferred output projections drip into the attention chunk loop as
    PE fillers placed between the score and attn@V matmuls.
  - Key-padding within the last compacted chunk is a per-partition
    bias of -30000 inside the exp activation, so padded lanes produce
    exactly 0. No max-subtraction pass: scores are ~N(0,1) after the
    1/sqrt(DH) scale.
"""

import math

import ml_dtypes
import numpy as np

B, S, D, H = 2, 2048, 1024, 16
DH = D // H            # 64
NCORES = 8
NG = 4                 # head-groups (cores per batch)
NH = H // NG           # 4 heads per core
MH = NH * DH           # 256: per-core slice of the projection space
PD = D // 128          # 8 contraction chunks for the projections
QT = 512               # q-tile width
NQT = S // QT          # 4 q tiles
SCALE = 1.0 / math.sqrt(DH)
BF16 = ml_dtypes.bfloat16

_NC_CACHE = {}


def _build_nc(nkv, dbg=False):
    """Build the (core-independent) Bass program for nkv key chunks."""
    if (nkv, dbg) in _NC_CACHE:
        return _NC_CACHE[(nkv, dbg)]

    from contextlib import ExitStack

    import concourse.bacc as bacc
    import concourse.mybir as mybir
    import concourse.tile as tile

    f32 = mybir.dt.float32
    bf16 = mybir.dt.bfloat16
    Exp = mybir.ActivationFunctionType.Exp

    nkvc = nkv * 128

    nc = bacc.Bacc("TRN2", target_bir_lowering=False, debug=False)

    xq = nc.dram_tensor("xq", [128, PD, S], bf16, kind="ExternalInput").ap()
    xk = nc.dram_tensor("xk", [128, PD, nkvc], bf16, kind="ExternalInput").ap()
    xv = nc.dram_tensor("xv", [128, PD, nkvc], bf16, kind="ExternalInput").ap()
    wq = nc.dram_tensor("wq", [128, PD, 2, 128], bf16, kind="ExternalInput").ap()
    wk = nc.dram_tensor("wk", [128, PD, 2, 128], bf16, kind="ExternalInput").ap()
    wv = nc.dram_tensor("wv", [128, PD, 2, 128], bf16, kind="ExternalInput").ap()
    wo = nc.dram_tensor("wo", [128, 2, D], bf16, kind="ExternalInput").ap()
    mb = nc.dram_tensor("mb", [128, nkv], f32, kind="ExternalInput").ap()
    # out[p, st, :] holds output row st*128 + p
    out = nc.dram_tensor("out", [128, NQT * 4, D], bf16,
                         kind="ExternalOutput").ap()
    if dbg:
        qdump = nc.dram_tensor("qdump", [128, 2, S], bf16,
                               kind="ExternalOutput").ap()
        kdump = nc.dram_tensor("kdump", [128, 2, nkv * 128], bf16,
                               kind="ExternalOutput").ap()
        vdump = nc.dram_tensor("vdump", [128, nkv, 2, 2, 66], bf16,
                               kind="ExternalOutput").ap()
        odump = nc.dram_tensor("odump", [128, 2, S], bf16,
                               kind="ExternalOutput").ap()
        otdump = nc.dram_tensor("otdump", [128, QT], f32,
                                kind="ExternalOutput").ap()
        atdump = nc.dram_tensor("atdump", [128, 2, QT], bf16,
                                kind="ExternalOutput").ap()
        rbdump = nc.dram_tensor("rbdump", [64, QT], f32,
                                kind="ExternalOutput").ap()

    # col-tile widths for the K/V projections over nkvc columns
    kv_tiles = []
    cs = 0
    while cs < nkvc:
        w = min(QT, nkvc - cs)
        kv_tiles.append((cs, w))
        cs += w

    with tile.TileContext(nc) as tc, ExitStack() as ctx:
        wpool = ctx.enter_context(tc.tile_pool(name="wpool", bufs=1))
        apool = ctx.enter_context(tc.tile_pool(name="apool", bufs=1))

        wq_sb = wpool.tile([128, PD, 2, 128], bf16)
        wk_sb = wpool.tile([128, PD, 2, 128], bf16)
        wv_sb = wpool.tile([128, PD, 2, 128], bf16)
        wo_sb = wpool.tile([128, 2, D], bf16)
        mb_sb = wpool.tile([128, nkv], f32)
        nc.sync.dma_start(wq_sb, wq)
        nc.sync.dma_start(mb_sb, mb)

        # Tiny warm-up ops: let DVE/ACT observe the mask DMA early and
        # pull the ~2.7us exp table load off the critical path.
        scratch = wpool.tile([1, 2], f32)
        nc.vector.tensor_copy(scratch, mb_sb[0:1, 0:2])
        scratch2 = wpool.tile([1, 2], f32)
        nc.scalar.activation(scratch2, mb_sb[0:1, 0:2], Exp)

        # Persistent per-core activations (partition = 2x64 head dims of
        # a head pair; the pair index is the second axis).
        xq_sb = apool.tile([128, PD, S], bf16)
        xk_sb = apool.tile([128, PD, nkvc], bf16)
        xv_sb = apool.tile([128, PD, nkvc], bf16)
        qT_sb = apool.tile([128, 2, S], bf16)
        kT_sb = apool.tile([128, 2, nkvc], bf16)
        # attn@V lhsT: per (pair, head) a 66-col sub-block [d0..d63, 1,
        # pad], so each head's 65-wide lhsT slice [V | 1] is contiguous
        # and Z lands on (aligned) PSUM partition 64.
        v65_sb = apool.tile([128, nkv, 2, 2, 66], bf16)
        nc.vector.memset(v65_sb, 1.0)
        otn_sb = apool.tile([128, 2, S], bf16)

        with (
            tc.tile_pool(name="atp", bufs=16) as atp,
            tc.tile_pool(name="rp", bufs=4) as rp,
            tc.tile_pool(name="rbp", bufs=4) as rbp,
            tc.tile_pool(name="outp", bufs=2) as outp,
            tc.tile_pool(name="pot", bufs=2, space="PSUM") as pot,
            tc.tile_pool(name="paux", bufs=2, space="PSUM") as paux,
            tc.tile_pool(name="pss", bufs=2, space="PSUM") as pss,
        ):
            # Input DMAs in consumption order, column-blocked so early
            # projection groups unblock as soon as their bytes land. One
            # coalesced DMA per (tensor, col-block): the [128, PD, cols]
            # DRAM layout matches the SBUF staging layout, so a whole
            # block moves in a single (cheap) HWDGE job.
            def dma_blk(x_sb, xd, cs, w):
                nc.sync.dma_start(x_sb[:, :, cs:cs + w], xd[:, :, cs:cs + w])

            dma_blk(xq_sb, xq, 0, QT)
            nc.sync.dma_start(wk_sb, wk)
            nc.sync.dma_start(wv_sb, wv)
            dma_blk(xk_sb, xk, *kv_tiles[0])
            dma_blk(xv_sb, xv, *kv_tiles[0])
            if len(kv_tiles) > 1:
                dma_blk(xk_sb, xk, *kv_tiles[1])
                dma_blk(xv_sb, xv, *kv_tiles[1])
            dma_blk(xq_sb, xq, QT, QT)
            for (cs, w) in kv_tiles[2:]:
                dma_blk(xk_sb, xk, cs, w)
                dma_blk(xv_sb, xv, cs, w)
            dma_blk(xq_sb, xq, 2 * QT, QT)
            dma_blk(xq_sb, xq, 3 * QT, QT)
            nc.sync.dma_start(wo_sb, wo)

            def proj(x_sb, w_sb, dst, t, cs, w):
                pq = paux.tile([128, QT], f32, tag="pq")
                for kc in range(PD):
                    nc.tensor.matmul(
                        pq[:, 0:w],
                        lhsT=w_sb[:, kc, t, :],
                        rhs=x_sb[:, kc, cs:cs + w],
                        start=(kc == 0),
                        stop=(kc == PD - 1),
                    )
                nc.vector.tensor_copy(dst[:, t, cs:cs + w], pq[:, 0:w])

            def projv(kc):
                # V projection for one 128-wide kv chunk, directly in the
                # [kvpos, dims] orientation (lhsT = the xv chunk itself),
                # so no transpose is needed: one DVE copy per pair drops
                # the [128, 128] pair block into the 66-col sub-blocks.
                pq = paux.tile([128, QT], f32, tag="pq")
                ks = kc * 128
                for kcin in range(PD):
                    nc.tensor.matmul(
                        pq[:, 0:MH],
                        lhsT=xv_sb[:, kcin, ks:ks + 128],
                        rhs=wv_sb[:, kcin, :, :],
                        start=(kcin == 0),
                        stop=(kcin == PD - 1),
                    )
                for t in range(2):
                    nc.vector.tensor_copy(
                        v65_sb[:, kc, t, :, 0:64],
                        pq[:, t * 128:(t + 1) * 128])

            ws4_box = [None]

            def emit_outproj_st(st, fine_dma=False):
                # Row-parallel output projection for one 128-row slice;
                # rows collect in a per-q-tile [128, 4, D] staging tile
                # that ships in one coalesced DMA (or per-slice DMAs for
                # the trailing q-tile, to shorten the kernel tail).
                rs = st * 128
                if st % 4 == 0:
                    ws4_box[0] = outp.tile([128, 4, D], bf16, tag="ws",
                                           name="ws4")
                ws4 = ws4_box[0]
                for nt in range(2):
                    wp = paux.tile([128, QT], f32, tag="pq")
                    for tt in range(2):
                        nc.tensor.matmul(
                            wp,
                            lhsT=otn_sb[:, tt, rs:rs + 128],
                            rhs=wo_sb[:, tt, nt * QT:(nt + 1) * QT],
                            start=(tt == 0), stop=(tt == 1),
                        )
                    nc.vector.tensor_copy(
                        ws4[:, st % 4, nt * QT:(nt + 1) * QT], wp)
                if fine_dma:
                    nc.sync.dma_start(out[:, st:st + 1, :],
                                      ws4[:, st % 4:st % 4 + 1, :])
                elif st % 4 == 3:
                    nc.sync.dma_start(out[:, st - 3:st + 1, :], ws4)

            # Pair-phased pipeline: a minimal prologue projects just what
            # the first attention chunks need; every other projection
            # group (pair 0 remainder, V chunks, then all of pair 1) is
            # drip-fed into the attention chunk stream as a PE filler, so
            # the (ACT-bound) exp stream starts as early as the input
            # DMAs allow and the PE stays fed. Pair 1's attention carries
            # the deferred output projections the same way, so the PE
            # never waits on a q-tile's normalize chain.
            proj(xq_sb, wq_sb, qT_sb, 0, 0, QT)
            proj(xk_sb, wk_sb, kT_sb, 0, *kv_tiles[0])
            projv(0)

            def F(f, *a):
                return lambda: f(*a)

            # Filler slots are global chunk indices; a slot's fillers are
            # emitted between that chunk's score and attn@V matmuls, so a
            # filler at slot c runs before av(chunk c) and before
            # sc(chunk c+1). Deadlines (all within pair-0 qt0):
            #   projv(kc): slot kc (just-in-time for its own av)
            #   k-proj col-tile j (cols cs..): slot cs//128 - 1
            #   q sti_i: slot i*nkv - 1 (first score of q-tile i)
            fill_at = {}

            def add_fill(slot, f):
                fill_at.setdefault(slot, []).append(f)

            for kc in range(1, nkv):
                add_fill(kc, F(projv, kc))
            for (cs, w) in kv_tiles[1:]:
                add_fill(cs // 128 - 1,
                         F(proj, xk_sb, wk_sb, kT_sb, 0, cs, w))
            for sti in range(1, NQT):
                add_fill(sti * nkv - 1,
                         F(proj, xq_sb, wq_sb, qT_sb, 0, sti * QT, QT))
            # pair-1 projections: anywhere before chunk 4*nkv (and their
            # own q-tile deadlines, which these early slots satisfy)
            t1_fills = (
                [F(proj, xk_sb, wk_sb, kT_sb, 1, cs, w) for (cs, w) in kv_tiles]
                + [F(proj, xq_sb, wq_sb, qT_sb, 1, sti * QT, QT)
                   for sti in range(NQT)]
            )
            for j, f in enumerate(t1_fills):
                add_fill(nkv + 2 + 2 * j, f)

            nchunk = 0
            # 4 evenly spread chunk indices at which to emit one deferred
            # output-projection slice during pair 1's attention
            op_marks = [1 + (j * (nkv - 2)) // 3 for j in range(4)]
            assert len(set(op_marks)) == 4
            for t in range(2):
                for qt in range(NQT):
                    qs = qt * QT
                    ota = pot.tile([128, QT], f32, tag="ot")
                    otb = pot.tile([128, QT], f32, tag="ot")
                    for kc in range(nkv):
                        ks = kc * 128
                        sc = pss.tile([128, 2, QT], f32, tag="sc")
                        nc.tensor.matmul(
                            sc[:, 0, :],
                            lhsT=kT_sb[0:DH, t, ks:ks + 128],
                            rhs=qT_sb[0:DH, t, qs:qs + QT],
                        )
                        nc.tensor.matmul(
                            sc[:, 1, :],
                            lhsT=kT_sb[DH:128, t, ks:ks + 128],
                            rhs=qT_sb[DH:128, t, qs:qs + QT],
                        )
                        attn = atp.tile([128, 2, QT], bf16, tag="attn")
                        nc.scalar.activation(attn, sc, Exp, scale=SCALE,
                                             bias=mb_sb[:, kc:kc + 1])
                        if dbg and qt == 0 and t == 0 and kc == 0:
                            nc.sync.dma_start(atdump, attn)
                        # PE fillers sit between the score and attn@V
                        # matmuls so they hide inside the exp wait
                        # instead of delaying the next chunk's scores:
                        # pair-1 projection groups during pair 0's
                        # attention, deferred output projections during
                        # pair 1's.
                        if t == 0:
                            for f in fill_at.get(nchunk, ()):
                                f()
                        elif qt >= 1 and kc in op_marks:
                            emit_outproj_st(
                                (qt - 1) * 4 + op_marks.index(kc))
                        nchunk += 1
                        # attn@V with Z folded in via the ones column:
                        # O^T in rows 0:64, Z in row 64.
                        nc.tensor.matmul(
                            ota[0:65, :], lhsT=v65_sb[:, kc, t, 0, 0:65],
                            rhs=attn[:, 0, :],
                            start=(kc == 0), stop=(kc == nkv - 1),
                            skip_group_check=True,
                        )
                        nc.tensor.matmul(
                            otb[0:65, :], lhsT=v65_sb[:, kc, t, 1, 0:65],
                            rhs=attn[:, 1, :],
                            start=(kc == 0), stop=(kc == nkv - 1),
                            skip_group_check=True,
                        )

                    if dbg and qt == 0 and t == 0:
                        otsb = rp.tile([128, QT], f32, tag="otd")
                        nc.vector.tensor_copy(otsb, ota)
                        nc.sync.dma_start(otdump, otsb)
                    for hodd, ot in ((0, ota), (1, otb)):
                        # Evacuate the accumulator to SBUF right away --
                        # the two copies are all that holds the PSUM
                        # bank, so the next q-tile's attn@V can start
                        # while the normalize chain runs from SBUF.
                        # (Z goes to partition 0 separately: custom-DVE
                        # ops mislower partition-offset reads.)
                        zc = rp.tile([1, QT], f32, tag="zc")
                        nc.vector.tensor_copy(zc, ot[64:65, :])
                        osb = rp.tile([64, QT], f32, tag="osb")
                        nc.vector.tensor_copy(osb, ot[0:64, :])
                        r = rp.tile([1, QT], f32, tag="r")
                        nc.vector.reciprocal_approx_fast(r, zc)
                        rb = rbp.tile([64, QT], f32, tag="rb")
                        nc.gpsimd.partition_broadcast(rb, r, channels=64)
                        if dbg and qt == 0 and t == 0 and hodd == 0:
                            nc.sync.dma_start(rbdump, rb)
                        nc.vector.tensor_mul(
                            otn_sb[hodd * 64:hodd * 64 + 64, t, qs:qs + QT],
                            osb, rb)
                # leftover fillers at end of pair-0 phase (slots past
                # the last pair-0 chunk, e.g. when nkv is small)
                if t == 0 and qt == NQT - 1:
                    for slot in sorted(fill_at):
                        if slot >= nchunk:
                            for f in fill_at[slot]:
                                f()
            for st in range((NQT - 1) * 4, NQT * 4):
                emit_outproj_st(st, fine_dma=True)

            if dbg:
                nc.sync.dma_start(qdump, qT_sb)
                nc.sync.dma_start(kdump, kT_sb)
                nc.sync.dma_start(vdump, v65_sb)
                nc.sync.dma_start(odump, otn_sb)

    nc.compile()
    _NC_CACHE[(nkv, dbg)] = nc
    return nc


def _prep_inputs(queries, keys, values, masks, Wq, Wk, Wv, Wo):
    """Host-side sharding/layout prep. Returns (nkv, per-core input maps)."""
    masks = np.asarray(masks)
    sels = [np.nonzero(masks[b] != 0)[0] for b in range(B)]
    ns = [len(s) for s in sels]
    nkv = max(1, (max(ns) + 127) // 128)
    nkvc = nkv * 128

    def t_bf16(x):  # [S, D] f32 -> [128, PD, S] bf16 contiguous
        xt = np.asarray(x, dtype=np.float32).astype(BF16).T  # [D, S]
        return np.ascontiguousarray(
            xt.reshape(PD, 128, -1).transpose(1, 0, 2))

    def t_comp(x, sel):  # gather valid rows, pad to nkvc, relayout
        xc = np.zeros((nkvc, D), dtype=BF16)
        xc[:len(sel)] = np.asarray(x, dtype=np.float32)[sel].astype(BF16)
        return np.ascontiguousarray(
            xc.T.reshape(PD, 128, nkvc).transpose(1, 0, 2))

    xq_b = [t_bf16(queries[b]) for b in range(B)]
    xk_b = [t_comp(keys[b], sels[b]) for b in range(B)]
    xv_b = [t_comp(values[b], sels[b]) for b in range(B)]

    mb_b = []
    for b in range(B):
        j = np.arange(nkvc).reshape(nkv, 128)
        mb_b.append(np.ascontiguousarray(
            np.where(j < ns[b], 0.0, -30000.0).astype(np.float32).T))

    def w_prep(W, g):  # [D, D] -> [128, PD, 2, 128] bf16 slice for group g
        Wg = np.asarray(W, dtype=np.float32)[:, g * MH:(g + 1) * MH]
        return np.ascontiguousarray(
            Wg.astype(BF16).reshape(PD, 128, 2, 128).transpose(1, 0, 2, 3))

    def wo_prep(W, g):  # [D, D] -> [128, 2, D] bf16 slice for group g
        Wg = np.asarray(W, dtype=np.float32)[g * MH:(g + 1) * MH, :]
        return np.ascontiguousarray(
            Wg.astype(BF16).reshape(2, 128, D).transpose(1, 0, 2))

    wq_g = [w_prep(Wq, g) for g in range(NG)]
    wk_g = [w_prep(Wk, g) for g in range(NG)]
    wv_g = [w_prep(Wv, g) for g in range(NG)]
    wo_g = [wo_prep(Wo, g) for g in range(NG)]

    in_maps = []
    for c in range(NCORES):
        b, g = c // NG, c % NG
        in_maps.append({
            "xq": xq_b[b], "xk": xk_b[b], "xv": xv_b[b],
            "wq": wq_g[g], "wk": wk_g[g], "wv": wv_g[g], "wo": wo_g[g],
            "mb": mb_b[b],
        })
    return nkv, in_maps


def run(inputs, trace=False, trace_cores=None):
    """Run on 8 NeuronCores; returns (output [B,S,D] f32, BassKernelResults)."""
    from concourse.bass_utils import run_bass_kernel_spmd

    nkv, in_maps = _prep_inputs(**inputs)
    nc = _build_nc(nkv)
    res = run_bass_kernel_spmd(
        nc, in_maps, core_ids=list(range(NCORES)),
        trace=trace, trace_cores=trace_cores,
    )
    out = np.empty((B, S, D), dtype=np.float32)
    for b in range(B):
        acc = res.results[b * NG]["out"].astype(np.float32)
        for g in range(1, NG):
            acc += res.results[b * NG + g]["out"].astype(np.float32)
        # [128, 16, D], row st*128+p at [p, st] -> [S, D]
        out[b] = acc.transpose(1, 0, 2).reshape(S, D)
    return out, res


def kernel(**inputs) -> np.ndarray:
    out, _ = run(inputs)
    return out


# revision 54
# speedup vs baseline: 2.0060x; 1.0137x over previous
"""Multi-head attention (B=2, S=2048, D=1024, H=16) on 8 Trainium2 cores.

Sharding: batch x head-quad. Core c owns batch c//4 and heads
{4g..4g+3} where g = c%4 (a contiguous 256-wide slice of the
projection space). Each core reads its batch's full queries plus the
mask-COMPACTED keys/values (masked-out key positions are dropped on
the host -- exact math, and it halves the attention work and the
key/value traffic), computes its 4 heads' Q/K/V projections, the
S x S_valid attention, and its partial contribution to the output
projection (row-parallel Wo). The host sums the 4 bf16 partials per
batch.

Device-side layout notes:
  - Scores are computed transposed (scoresT[k, q]) so the softmax
    contraction (over k) lands on the PSUM partition axis. The two
    K=64 score matmuls of a head pair sit on SBUF partitions 0:64 and
    64:128, so bass's inferred tile_position row-tiles them into
    concurrent halves of the PE array on hardware.
  - V is projected directly in [kvpos, dims] orientation (lhsT = the
    xv chunk itself), so no transposes are needed anywhere.
  - The softmax denominator Z is folded into the attn@V matmul: each
    head's lhsT is a contiguous 65-wide [V | 1] slice (66-col
    sub-blocks of the v65 tile), so Z lands on (aligned) PSUM
    partition 64 of the same accumulator -- no separate Z matmuls.
    1/Z is broadcast to the head's 64 partitions with a GPSIMD
    partition_broadcast (the engine is otherwise idle) and applied
    with one DVE multiply, after a DVE evacuation of the accumulator
    to SBUF that frees the PSUM bank for the next q-tile.
  - The whole program is software-pipelined around the ACT-bound exp
    stream: a minimal prologue, then every projection group and the
    deferred output projections drip into the attention chunk loop as
    PE fillers placed between the score and attn@V matmuls.
  - Key-padding within the last compacted chunk is a per-partition
    bias of -30000 inside the exp activation, so padded lanes produce
    exactly 0. No max-subtraction pass: scores are ~N(0,1) after the
    1/sqrt(DH) scale.
"""

import math

import ml_dtypes
import numpy as np

B, S, D, H = 2, 2048, 1024, 16
DH = D // H            # 64
NCORES = 8
NG = 4                 # head-groups (cores per batch)
NH = H // NG           # 4 heads per core
MH = NH * DH           # 256: per-core slice of the projection space
PD = D // 128          # 8 contraction chunks for the projections
QT = 512               # q-tile width
NQT = S // QT          # 4 q tiles
SCALE = 1.0 / math.sqrt(DH)
BF16 = ml_dtypes.bfloat16

_NC_CACHE = {}


def _build_nc(nkv, dbg=False):
    """Build the (core-independent) Bass program for nkv key chunks."""
    if (nkv, dbg) in _NC_CACHE:
        return _NC_CACHE[(nkv, dbg)]

    from contextlib import ExitStack

    import concourse.bacc as bacc
    import concourse.mybir as mybir
    import concourse.tile as tile

    f32 = mybir.dt.float32
    bf16 = mybir.dt.bfloat16
    Exp = mybir.ActivationFunctionType.Exp

    nkvc = nkv * 128

    nc = bacc.Bacc("TRN2", target_bir_lowering=False, debug=False)

    xq = nc.dram_tensor("xq", [128, PD, S], bf16, kind="ExternalInput").ap()
    xk = nc.dram_tensor("xk", [128, PD, nkvc], bf16, kind="ExternalInput").ap()
    xv = nc.dram_tensor("xv", [128, PD, nkvc], bf16, kind="ExternalInput").ap()
    wq = nc.dram_tensor("wq", [128, PD, 2, 128], bf16, kind="ExternalInput").ap()
    wk = nc.dram_tensor("wk", [128, PD, 2, 128], bf16, kind="ExternalInput").ap()
    wv = nc.dram_tensor("wv", [128, PD, 2, 128], bf16, kind="ExternalInput").ap()
    wo = nc.dram_tensor("wo", [128, 2, D], bf16, kind="ExternalInput").ap()
    mb = nc.dram_tensor("mb", [128, nkv], f32, kind="ExternalInput").ap()
    # out[p, st, :] holds output row st*128 + p
    out = nc.dram_tensor("out", [128, NQT * 4, D], bf16,
                         kind="ExternalOutput").ap()
    if dbg:
        qdump = nc.dram_tensor("qdump", [128, 2, S], bf16,
                               kind="ExternalOutput").ap()
        kdump = nc.dram_tensor("kdump", [128, 2, nkv * 128], bf16,
                               kind="ExternalOutput").ap()
        vdump = nc.dram_tensor("vdump", [128, nkv, 2, 2, 66], bf16,
                               kind="ExternalOutput").ap()
        odump = nc.dram_tensor("odump", [128, 2, S], bf16,
                               kind="ExternalOutput").ap()
        otdump = nc.dram_tensor("otdump", [128, QT], f32,
                                kind="ExternalOutput").ap()
        atdump = nc.dram_tensor("atdump", [128, 2, QT], bf16,
                                kind="ExternalOutput").ap()
        rbdump = nc.dram_tensor("rbdump", [64, QT], f32,
                                kind="ExternalOutput").ap()

    # col-tile widths for the K/V projections over nkvc columns
    kv_tiles = []
    cs = 0
    while cs < nkvc:
        w = min(QT, nkvc - cs)
        kv_tiles.append((cs, w))
        cs += w

    with tile.TileContext(nc) as tc, ExitStack() as ctx:
        wpool = ctx.enter_context(tc.tile_pool(name="wpool", bufs=1))
        apool = ctx.enter_context(tc.tile_pool(name="apool", bufs=1))

        wq_sb = wpool.tile([128, PD, 2, 128], bf16)
        wk_sb = wpool.tile([128, PD, 2, 128], bf16)
        wv_sb = wpool.tile([128, PD, 2, 128], bf16)
        wo_sb = wpool.tile([128, 2, D], bf16)
        mb_sb = wpool.tile([128, nkv], f32)
        nc.sync.dma_start(wq_sb, wq)
        nc.sync.dma_start(mb_sb, mb)

        # Tiny warm-up ops: let DVE/ACT observe the mask DMA early and
        # pull the ~2.7us exp table load off the critical path.
        scratch = wpool.tile([1, 2], f32)
        nc.vector.tensor_copy(scratch, mb_sb[0:1, 0:2])
        scratch2 = wpool.tile([1, 2], f32)
        nc.scalar.activation(scratch2, mb_sb[0:1, 0:2], Exp)

        # Persistent per-core activations (partition = 2x64 head dims of
        # a head pair; the pair index is the second axis).
        xq_sb = apool.tile([128, PD, S], bf16)
        xk_sb = apool.tile([128, PD, nkvc], bf16)
        xv_sb = apool.tile([128, PD, nkvc], bf16)
        qT_sb = apool.tile([128, 2, S], bf16)
        kT_sb = apool.tile([128, 2, nkvc], bf16)
        # attn@V lhsT: per (pair, head) a 66-col sub-block [d0..d63, 1,
        # pad], so each head's 65-wide lhsT slice [V | 1] is contiguous
        # and Z lands on (aligned) PSUM partition 64.
        v65_sb = apool.tile([128, nkv, 2, 2, 66], bf16)
        nc.vector.memset(v65_sb, 1.0)
        otn_sb = apool.tile([128, 2, S], bf16)

        with (
            tc.tile_pool(name="atp", bufs=16) as atp,
            tc.tile_pool(name="rp", bufs=4) as rp,
            tc.tile_pool(name="rbp", bufs=4) as rbp,
            tc.tile_pool(name="outp", bufs=2) as outp,
            tc.tile_pool(name="pot", bufs=2, space="PSUM") as pot,
            tc.tile_pool(name="paux", bufs=2, space="PSUM") as paux,
            tc.tile_pool(name="pss", bufs=2, space="PSUM") as pss,
        ):
            # Input DMAs in consumption order, column-blocked so early
            # projection groups unblock as soon as their bytes land. One
            # coalesced DMA per (tensor, col-block): the [128, PD, cols]
            # DRAM layout matches the SBUF staging layout, so a whole
            # block moves in a single (cheap) HWDGE job.
            def dma_blk(x_sb, xd, cs, w):
                nc.sync.dma_start(x_sb[:, :, cs:cs + w], xd[:, :, cs:cs + w])

            dma_blk(xq_sb, xq, 0, QT)
            nc.sync.dma_start(wk_sb, wk)
            nc.sync.dma_start(wv_sb, wv)
            dma_blk(xk_sb, xk, *kv_tiles[0])
            dma_blk(xv_sb, xv, *kv_tiles[0])
            if len(kv_tiles) > 1:
                dma_blk(xk_sb, xk, *kv_tiles[1])
                dma_blk(xv_sb, xv, *kv_tiles[1])
            dma_blk(xq_sb, xq, QT, QT)
            for (cs, w) in kv_tiles[2:]:
                dma_blk(xk_sb, xk, cs, w)
                dma_blk(xv_sb, xv, cs, w)
            dma_blk(xq_sb, xq, 2 * QT, QT)
            dma_blk(xq_sb, xq, 3 * QT, QT)
            nc.sync.dma_start(wo_sb, wo)

            def proj(x_sb, w_sb, dst, t, cs, w):
                pq = paux.tile([128, QT], f32, tag="pq")
                for kc in range(PD):
                    nc.tensor.matmul(
                        pq[:, 0:w],
                        lhsT=w_sb[:, kc, t, :],
                        rhs=x_sb[:, kc, cs:cs + w],
                        start=(kc == 0),
                        stop=(kc == PD - 1),
                    )
                nc.vector.tensor_copy(dst[:, t, cs:cs + w], pq[:, 0:w])

            def projv(kc):
                # V projection for one 128-wide kv chunk, directly in the
                # [kvpos, dims] orientation (lhsT = the xv chunk itself),
                # so no transpose is needed: one DVE copy per pair drops
                # the [128, 128] pair block into the 66-col sub-blocks.
                pq = paux.tile([128, QT], f32, tag="pq")
                ks = kc * 128
                for kcin in range(PD):
                    nc.tensor.matmul(
                        pq[:, 0:MH],
                        lhsT=xv_sb[:, kcin, ks:ks + 128],
                        rhs=wv_sb[:, kcin, :, :],
                        start=(kcin == 0),
                        stop=(kcin == PD - 1),
                    )
                for t in range(2):
                    nc.vector.tensor_copy(
                        v65_sb[:, kc, t, :, 0:64],
                        pq[:, t * 128:(t + 1) * 128])

            ws4_box = [None]

            def emit_outproj_st(st, fine_dma=False):
                # Row-parallel output projection for one 128-row slice;
                # rows collect in a per-q-tile [128, 4, D] staging tile
                # that ships in one coalesced DMA (or per-slice DMAs for
                # the trailing q-tile, to shorten the kernel tail).
                rs = st * 128
                if st % 4 == 0:
                    ws4_box[0] = outp.tile([128, 4, D], bf16, tag="ws",
                                           name="ws4")
                ws4 = ws4_box[0]
                for nt in range(2):
                    wp = paux.tile([128, QT], f32, tag="pq")
                    for tt in range(2):
                        nc.tensor.matmul(
                            wp,
                            lhsT=otn_sb[:, tt, rs:rs + 128],
                            rhs=wo_sb[:, tt, nt * QT:(nt + 1) * QT],
                            start=(tt == 0), stop=(tt == 1),
                        )
                    nc.vector.tensor_copy(
                        ws4[:, st % 4, nt * QT:(nt + 1) * QT], wp)
                if fine_dma:
                    nc.sync.dma_start(out[:, st:st + 1, :],
                                      ws4[:, st % 4:st % 4 + 1, :])
                elif st % 4 == 3:
                    nc.sync.dma_start(out[:, st - 3:st + 1, :], ws4)

            # Pair-phased pipeline: a minimal prologue projects just what
            # the first attention chunks need; every other projection
            # group (pair 0 remainder, V chunks, then all of pair 1) is
            # drip-fed into the attention chunk stream as a PE filler, so
            # the (ACT-bound) exp stream starts as early as the input
            # DMAs allow and the PE stays fed. Pair 1's attention carries
            # the deferred output projections the same way, so the PE
            # never waits on a q-tile's normalize chain.
            proj(xq_sb, wq_sb, qT_sb, 0, 0, QT)
            proj(xk_sb, wk_sb, kT_sb, 0, *kv_tiles[0])
            projv(0)

            def F(f, *a):
                return lambda: f(*a)

            # Filler slots are global chunk indices; a slot's fillers are
            # emitted between that chunk's score and attn@V matmuls, so a
            # filler at slot c runs before av(chunk c) and before
            # sc(chunk c+1). Deadlines (all within pair-0 qt0):
            #   projv(kc): slot kc (just-in-time for its own av)
            #   k-proj col-tile j (cols cs..): slot cs//128 - 1
            #   q sti_i: slot i*nkv - 1 (first score of q-tile i)
            fill_at = {}

            def add_fill(slot, f):
                fill_at.setdefault(slot, []).append(f)

            for kc in range(1, nkv):
                add_fill(kc, F(projv, kc))
            for (cs, w) in kv_tiles[1:]:
                add_fill(cs // 128 - 1,
                         F(proj, xk_sb, wk_sb, kT_sb, 0, cs, w))
            for sti in range(1, NQT):
                add_fill(sti * nkv - 1,
                         F(proj, xq_sb, wq_sb, qT_sb, 0, sti * QT, QT))
            # pair-1 projections: anywhere before chunk 4*nkv (and their
            # own q-tile deadlines, which these early slots satisfy)
            t1_fills = (
                [F(proj, xk_sb, wk_sb, kT_sb, 1, cs, w) for (cs, w) in kv_tiles]
                + [F(proj, xq_sb, wq_sb, qT_sb, 1, sti * QT, QT)
                   for sti in range(NQT)]
            )
            for j, f in enumerate(t1_fills):
                add_fill(nkv + 2 + 2 * j, f)

            nchunk = 0
            # 4 evenly spread chunk indices at which to emit one deferred
            # output-projection slice during pair 1's attention
            op_marks = [1 + (j * (nkv - 2)) // 3 for j in range(4)]
            assert len(set(op_marks)) == 4
            for t in range(2):
                for qt in range(NQT):
                    qs = qt * QT
                    ota = pot.tile([128, QT], f32, tag="ot")
                    otb = pot.tile([128, QT], f32, tag="ot")
                    for kc in range(nkv):
                        ks = kc * 128
                        sc = pss.tile([128, 2, QT], f32, tag="sc")
                        nc.tensor.matmul(
                            sc[:, 0, :],
                            lhsT=kT_sb[0:DH, t, ks:ks + 128],
                            rhs=qT_sb[0:DH, t, qs:qs + QT],
                        )
                        nc.tensor.matmul(
                            sc[:, 1, :],
                            lhsT=kT_sb[DH:128, t, ks:ks + 128],
                            rhs=qT_sb[DH:128, t, qs:qs + QT],
                        )
                        attn = atp.tile([128, 2, QT], bf16, tag="attn")
                        nc.scalar.activation(attn, sc, Exp, scale=SCALE,
                                             bias=mb_sb[:, kc:kc + 1])
                        if dbg and qt == 0 and t == 0 and kc == 0:
                            nc.sync.dma_start(atdump, attn)
                        # PE fillers sit between the score and attn@V
                        # matmuls so they hide inside the exp wait
                        # instead of delaying the next chunk's scores:
                        # pair-1 projection groups during pair 0's
                        # attention, deferred output projections during
                        # pair 1's.
                        if t == 0:
                            for f in fill_at.get(nchunk, ()):
                                f()
                        elif qt >= 1 and kc in op_marks:
                            emit_outproj_st(
                                (qt - 1) * 4 + op_marks.index(kc))
                        nchunk += 1
                        # attn@V with Z folded in via the ones column:
                        # O^T in rows 0:64, Z in row 64.
                        nc.tensor.matmul(
                            ota[0:65, :], lhsT=v65_sb[:, kc, t, 0, 0:65],
                            rhs=attn[:, 0, :],
                            start=(kc == 0), stop=(kc == nkv - 1),
                            skip_group_check=True,
                        )
                        nc.tensor.matmul(
                            otb[0:65, :], lhsT=v65_sb[:, kc, t, 1, 0:65],
                            rhs=attn[:, 1, :],
                            start=(kc == 0), stop=(kc == nkv - 1),
                            skip_group_check=True,
                        )

                    if dbg and qt == 0 and t == 0:
                        otsb = rp.tile([128, QT], f32, tag="otd")
                        nc.vector.tensor_copy(otsb, ota)
                        nc.sync.dma_start(otdump, otsb)
                    last_qt = t == 1 and qt == NQT - 1
                    for hodd, ot in ((0, ota), (1, otb)):
                        # Evacuate the accumulator to SBUF right away --
                        # the two copies are all that holds the PSUM
                        # bank, so the next q-tile's attn@V can start
                        # while the normalize chain runs from SBUF.
                        # (Z goes to partition 0 separately: custom-DVE
                        # ops mislower partition-offset reads.) The last
                        # q-tile has no successor, so it skips the
                        # evacuation and multiplies straight from PSUM.
                        zc = rp.tile([1, QT], f32, tag="zc")
                        nc.vector.tensor_copy(zc, ot[64:65, :])
                        if last_qt:
                            osb = ot[0:64, :]
                        else:
                            osb = rp.tile([64, QT], f32, tag="osb")
                            nc.vector.tensor_copy(osb, ot[0:64, :])
                        r = rp.tile([1, QT], f32, tag="r")
                        nc.vector.reciprocal_approx_fast(r, zc)
                        rb = rbp.tile([64, QT], f32, tag="rb")
                        nc.gpsimd.partition_broadcast(rb, r, channels=64)
                        if dbg and qt == 0 and t == 0 and hodd == 0:
                            nc.sync.dma_start(rbdump, rb)
                        nc.vector.tensor_mul(
                            otn_sb[hodd * 64:hodd * 64 + 64, t, qs:qs + QT],
                            osb, rb)
                # leftover fillers at end of pair-0 phase (slots past
                # the last pair-0 chunk, e.g. when nkv is small)
                if t == 0 and qt == NQT - 1:
                    for slot in sorted(fill_at):
                        if slot >= nchunk:
                            for f in fill_at[slot]:
                                f()
            for st in range((NQT - 1) * 4, NQT * 4):
                emit_outproj_st(st, fine_dma=True)

            if dbg:
                nc.sync.dma_start(qdump, qT_sb)
                nc.sync.dma_start(kdump, kT_sb)
                nc.sync.dma_start(vdump, v65_sb)
                nc.sync.dma_start(odump, otn_sb)

    nc.compile()
    _NC_CACHE[(nkv, dbg)] = nc
    return nc


def _prep_inputs(queries, keys, values, masks, Wq, Wk, Wv, Wo):
    """Host-side sharding/layout prep. Returns (nkv, per-core input maps)."""
    masks = np.asarray(masks)
    sels = [np.nonzero(masks[b] != 0)[0] for b in range(B)]
    ns = [len(s) for s in sels]
    nkv = max(1, (max(ns) + 127) // 128)
    nkvc = nkv * 128

    def t_bf16(x):  # [S, D] f32 -> [128, PD, S] bf16 contiguous
        xt = np.asarray(x, dtype=np.float32).astype(BF16).T  # [D, S]
        return np.ascontiguousarray(
            xt.reshape(PD, 128, -1).transpose(1, 0, 2))

    def t_comp(x, sel):  # gather valid rows, pad to nkvc, relayout
        xc = np.zeros((nkvc, D), dtype=BF16)
        xc[:len(sel)] = np.asarray(x, dtype=np.float32)[sel].astype(BF16)
        return np.ascontiguousarray(
            xc.T.reshape(PD, 128, nkvc).transpose(1, 0, 2))

    xq_b = [t_bf16(queries[b]) for b in range(B)]
    xk_b = [t_comp(keys[b], sels[b]) for b in range(B)]
    xv_b = [t_comp(values[b], sels[b]) for b in range(B)]

    mb_b = []
    for b in range(B):
        j = np.arange(nkvc).reshape(nkv, 128)
        mb_b.append(np.ascontiguousarray(
            np.where(j < ns[b], 0.0, -30000.0).astype(np.float32).T))

    def w_prep(W, g):  # [D, D] -> [128, PD, 2, 128] bf16 slice for group g
        Wg = np.asarray(W, dtype=np.float32)[:, g * MH:(g + 1) * MH]
        return np.ascontiguousarray(
            Wg.astype(BF16).reshape(PD, 128, 2, 128).transpose(1, 0, 2, 3))

    def wo_prep(W, g):  # [D, D] -> [128, 2, D] bf16 slice for group g
        Wg = np.asarray(W, dtype=np.float32)[g * MH:(g + 1) * MH, :]
        return np.ascontiguousarray(
            Wg.astype(BF16).reshape(2, 128, D).transpose(1, 0, 2))

    wq_g = [w_prep(Wq, g) for g in range(NG)]
    wk_g = [w_prep(Wk, g) for g in range(NG)]
    wv_g = [w_prep(Wv, g) for g in range(NG)]
    wo_g = [wo_prep(Wo, g) for g in range(NG)]

    in_maps = []
    for c in range(NCORES):
        b, g = c // NG, c % NG
        in_maps.append({
            "xq": xq_b[b], "xk": xk_b[b], "xv": xv_b[b],
            "wq": wq_g[g], "wk": wk_g[g], "wv": wv_g[g], "wo": wo_g[g],
            "mb": mb_b[b],
        })
    return nkv, in_maps


def run(inputs, trace=False, trace_cores=None):
    """Run on 8 NeuronCores; returns (output [B,S,D] f32, BassKernelResults)."""
    from concourse.bass_utils import run_bass_kernel_spmd

    nkv, in_maps = _prep_inputs(**inputs)
    nc = _build_nc(nkv)
    res = run_bass_kernel_spmd(
        nc, in_maps, core_ids=list(range(NCORES)),
        trace=trace, trace_cores=trace_cores,
    )
    out = np.empty((B, S, D), dtype=np.float32)
    for b in range(B):
        acc = res.results[b * NG]["out"].astype(np.float32)
        for g in range(1, NG):
            acc += res.results[b * NG + g]["out"].astype(np.float32)
        # [128, 16, D], row st*128+p at [p, st] -> [S, D]
        out[b] = acc.transpose(1, 0, 2).reshape(S, D)
    return out, res


def kernel(**inputs) -> np.ndarray:
    out, _ = run(inputs)
    return out
